# revision 12
# baseline (speedup 1.0000x reference)
# GCN-conv kernel for Trainium2 (Bass/Tile), 8-core SPMD.
#
# Reference computation:
#   A   = segment_sum(edge_attr[E,64], row_idx, N)          # scatter-add
#   out = relu((A / D[:,None]) @ W_pass + b_pass + X @ W_self + b_self)
#
# Sharding strategy (host-side prep, hardcoded for the 50000/1.6M problem):
#   * Nodes are assigned to 8*49 = 392 bins of <=128 "lanes" each via a
#     degree-balanced snake assignment, so each bin owns ~4080 edges.
#     Bin b -> (core b//49, window b%49).  Each core owns 49 windows
#     (6272 node slots) and ONLY the edges targeting its own nodes, so no
#     cross-core reduction is needed.
#   * Edges are bucketed by bin.  Each 128-edge chunk targets a single
#     128-lane window.  Per chunk the device builds a one-hot matrix
#     onehot[e, lane] = (lidx[e] == lane) on the vector engine and the
#     tensor engine computes  edge_chunk.T @ onehot  accumulated in PSUM
#     over the window's chunks -> per-window aggregate [64, 128].
#   * fp32 edge values are split on the host into bf16 hi + bf16 lo parts
#     (exact to ~2^-17), laid out as 128 stationary columns (64 hi | 64 lo)
#     so ONE bf16 matmul per chunk computes both partial sums at full PE
#     rate; the two halves are folded (and scaled by 1/D) when copying
#     PSUM -> SBUF.
#   * X is pre-transposed on the host; final linears run per window:
#     psum[128n, 256] = A_w.T@W_pass + XT_w.T@W_self + 1.T@bias, then ReLU
#     on the scalar engine and a direct DMA store.
#   * Output rows come back in slot order and are un-permuted on the host.

import math
import os
import tempfile
from types import SimpleNamespace

import numpy as np
import ml_dtypes

import concourse.bass as bass
import concourse.mybir as mybir
import concourse.tile as tile
from concourse import bacc
from concourse.bass import ts
from concourse.bass_utils import run_bass_kernel_spmd

BF16 = ml_dtypes.bfloat16

# problem dims (hardcoded per contract)
N_NODES = 50000
N_EDGES = 1600000
CE = 64
CN = 256
CO = 256
N_CORES = 8
WIN = 128  # nodes per window (PSUM free dim of the segment matmul)

# final-linear matmul dtype: fp32 is exact but 4 cyc/row on PE; float32r is
# full rate at N>=256 but with hardware-specific numerics (validated on HW
# before enabling).
USE_F32R = False

_F32R = mybir.dt.float32r


def _make_plan(row_idx, n_nodes, n_cores):
    """Host-side layout plan. Pure numpy, no device work."""
    row_idx = np.asarray(row_idx).astype(np.int64)
    E = row_idx.shape[0]
    wpc = math.ceil(n_nodes / (WIN * n_cores))  # windows per core
    nbins = wpc * n_cores
    slots_per_core = wpc * WIN

    deg = np.bincount(row_idx, minlength=n_nodes)
    # snake assignment of nodes (sorted by degree desc) into bins ->
    # per-bin edge counts are near-equal.
    order_nodes = np.argsort(-deg, kind="stable")
    ranks = np.arange(n_nodes)
    r, j = ranks // nbins, ranks % nbins
    binid_for_rank = np.where(r % 2 == 0, j, nbins - 1 - j)
    node_bin = np.empty(n_nodes, np.int64)
    node_bin[order_nodes] = binid_for_rank

    # lane within bin
    nodes_sorted = np.argsort(node_bin, kind="stable")
    bin_ncnt = np.bincount(node_bin, minlength=nbins)
    assert bin_ncnt.max() <= WIN, f"bin overflow: {bin_ncnt.max()} nodes"
    nstarts = np.concatenate([[0], np.cumsum(bin_ncnt)[:-1]])
    lane_sorted = np.arange(n_nodes) - np.repeat(nstarts, bin_ncnt)
    node_lane = np.empty(n_nodes, np.int64)
    node_lane[nodes_sorted] = lane_sorted

    node_slot = node_bin * WIN + node_lane  # node -> global slot
    slot_node = np.full(nbins * WIN, -1, np.int64)
    slot_node[node_slot] = np.arange(n_nodes)

    # edges -> (bin, lane)
    ebin = node_bin[row_idx]
    elane = node_lane[row_idx]
    ecnt = np.bincount(ebin, minlength=nbins).reshape(n_cores, wpc)
    K = np.maximum(1, np.ceil(ecnt.max(axis=0) / 128.0)).astype(np.int64)  # [wpc]
    # pad total chunk count to a multiple of 8 (edge DMA tile = 8 chunks)
    K[-1] += (-K.sum()) % 8
    C_total = int(K.sum())
    G = C_total // 8
    caps = K * 128
    woff = np.concatenate([[0], np.cumsum(caps)[:-1]])  # chunk-slot offset per window

    # position of each edge inside its core's padded edge buffer
    eorder = np.argsort(ebin, kind="stable")
    ebin_s = ebin[eorder]
    ecnt_flat = np.bincount(ebin, minlength=nbins)
    estarts = np.concatenate([[0], np.cumsum(ecnt_flat)[:-1]])
    rank_in_bin = np.arange(E) - np.repeat(estarts, ecnt_flat)
    w_of = ebin_s % wpc
    core_of = ebin_s // wpc
    pos_in_core = woff[w_of] + rank_in_bin

    return SimpleNamespace(
        n_nodes=n_nodes,
        n_cores=n_cores,
        wpc=wpc,
        nbins=nbins,
        slots_per_core=slots_per_core,
        K=K,
        C_total=C_total,
        G=G,
        slot_node=slot_node,
        eorder=eorder,
        core_of=core_of,
        pos_in_core=pos_in_core,
        elane=elane,
    )


def _prep_inputs(inputs, plan):
    """Build the 8 per-core input maps (and shared constants)."""
    p = plan
    edge_attr = np.ascontiguousarray(np.asarray(inputs["edge_attr"], np.float32))
    X = np.asarray(inputs["X"], np.float32)
    D = np.asarray(inputs["D"], np.float32)
    Wp = np.ascontiguousarray(np.asarray(inputs["W_pass"], np.float32))
    Ws = np.ascontiguousarray(np.asarray(inputs["W_self"], np.float32))
    bp = np.asarray(inputs["b_pass"], np.float32).reshape(1, CO)
    bs = np.asarray(inputs["b_self"], np.float32).reshape(1, CO)

    EB = p.C_total * 128  # edge slots per core

    ehi = edge_attr.astype(BF16)
    elo = (edge_attr - ehi.astype(np.float32)).astype(BF16)

    src = p.eorder
    ebuf = np.zeros((p.n_cores, EB, 2 * CE), BF16)
    ebuf[p.core_of, p.pos_in_core, 0:CE] = ehi[src]
    ebuf[p.core_of, p.pos_in_core, CE : 2 * CE] = elo[src]
    # [core, chunk, lane, 128] -> [core, G, 128(lane), 8(sub), 128]
    ebuf = ebuf.reshape(p.n_cores, p.G, 8, 128, 2 * CE).transpose(0, 1, 3, 2, 4)
    ebuf = np.ascontiguousarray(ebuf).reshape(p.n_cores, p.G, 128, 8 * 2 * CE)

    lidx = np.zeros((p.n_cores, EB), np.float32)
    lidx[p.core_of, p.pos_in_core] = p.elane[src].astype(np.float32)
    lidxT = np.ascontiguousarray(
        lidx.reshape(p.n_cores, p.C_total, 128).transpose(0, 2, 1)
    )

    nslots = p.nbins * WIN
    valid = p.slot_node >= 0
    XT_all = np.zeros((CN, nslots), np.float32)
    XT_all[:, valid] = X.T[:, p.slot_node[valid]]
    D_all = np.ones(nslots, np.float32)
    D_all[valid] = D[p.slot_node[valid]]

    iota_t = np.tile(np.arange(128, dtype=np.float32), (128, 1)).astype(BF16)
    # W_pass stacked twice: the final matmul contracts the (hi|lo)-stacked
    # aggregate over K=128, folding hi+lo for free.
    Wp2 = np.ascontiguousarray(np.vstack([Wp, Wp]))

    in_maps = []
    for c in range(p.n_cores):
        s0, s1 = c * p.slots_per_core, (c + 1) * p.slots_per_core
        db = np.broadcast_to(D_all[s0:s1], (128, p.slots_per_core))
        in_maps.append(
            {
                "edges": ebuf[c],
                "lidx": lidxT[c],
                "iota": iota_t,
                "xt": np.ascontiguousarray(XT_all[:, s0:s1]),
                "db": np.ascontiguousarray(db),
                "wp": Wp2,
                "ws": Ws,
                "bp": bp,
                "bs": bs,
            }
        )
    return in_maps


def _build_nc(plan, repeat=1):
    """Emit the Bass/Tile program (same structure for all cores).

    repeat > 1 wraps the main loop in a hardware For loop that recomputes the
    same result `repeat` times — used only for benchmarking (amortizes the
    host->device dispatch overhead of the measurement harness).
    """
    p = plan
    SPC = p.slots_per_core
    f32 = mybir.dt.float32
    bf16 = mybir.dt.bfloat16

    nc = bacc.Bacc(
        "TRN2",
        target_bir_lowering=False,
        debug=False,
        num_devices=p.n_cores,
    )

    edges_t = nc.dram_tensor("edges", [p.G, 128, 8 * 2 * CE], bf16, kind="ExternalInput")
    lidx_t = nc.dram_tensor("lidx", [128, p.C_total], f32, kind="ExternalInput")
    iota_t = nc.dram_tensor("iota", [128, 128], bf16, kind="ExternalInput")
    xt_t = nc.dram_tensor("xt", [CN, SPC], f32, kind="ExternalInput")
    db_t = nc.dram_tensor("db", [128, SPC], f32, kind="ExternalInput")
    wp_t = nc.dram_tensor("wp", [2 * CE, CO], f32, kind="ExternalInput")
    ws_t = nc.dram_tensor("ws", [CN, CO], f32, kind="ExternalInput")
    bp_t = nc.dram_tensor("bp", [1, CO], f32, kind="ExternalInput")
    bs_t = nc.dram_tensor("bs", [1, CO], f32, kind="ExternalInput")
    out_t = nc.dram_tensor("out", [SPC, CO], f32, kind="ExternalOutput")

    def mmdt(ap):
        return ap.bitcast(_F32R) if USE_F32R else ap

    with tile.TileContext(nc, num_cores=p.n_cores) as tc:
        with (
            tc.tile_pool(name="const", bufs=1) as const_p,
            tc.tile_pool(name="big", bufs=1) as big_p,
            tc.tile_pool(name="epool", bufs=3) as epool,
            tc.tile_pool(name="ohpool", bufs=8) as ohpool,
            tc.tile_pool(name="outp", bufs=3) as outp,
            tc.tile_pool(name="psw", bufs=2, space="PSUM") as psw_p,
            tc.tile_pool(name="pso", bufs=2, space="PSUM") as pso_p,
        ):
            # ---- constants / persistent tensors ----
            iota_sb = const_p.tile([128, 128], bf16)
            nc.scalar.dma_start(iota_sb[:], iota_t[:])
            wp_sb = const_p.tile([2 * CE, CO], f32)
            nc.scalar.dma_start(wp_sb[:], wp_t[:])
            ws0_sb = const_p.tile([128, CO], f32)
            ws1_sb = const_p.tile([128, CO], f32)
            nc.scalar.dma_start(ws0_sb[:], ws_t[0:128, :])
            nc.scalar.dma_start(ws1_sb[:], ws_t[128:256, :])
            bpv = const_p.tile([1, CO], f32)
            bsv = const_p.tile([1, CO], f32)
            nc.scalar.dma_start(bpv[:], bp_t[:])
            nc.scalar.dma_start(bsv[:], bs_t[:])
            bias_sb = const_p.tile([1, CO], f32)
            nc.vector.tensor_add(bias_sb[:], bpv[:], bsv[:])
            ones_sb = const_p.tile([1, 128], f32)
            nc.vector.memset(ones_sb[:], 1.0)

            lidx_sb = big_p.tile([128, p.C_total], f32)
            nc.scalar.dma_start(lidx_sb[:], lidx_t[:])
            xt0_sb = big_p.tile([128, SPC], f32)
            xt1_sb = big_p.tile([128, SPC], f32)
            nc.scalar.dma_start(xt0_sb[:], xt_t[0:128, :])
            nc.scalar.dma_start(xt1_sb[:], xt_t[128:256, :])

            # 1/D broadcast across all 128 (hi|lo)-stacked partitions
            recip_sb = big_p.tile([128, SPC], f32)
            nc.scalar.dma_start(recip_sb[:], db_t[:])
            rscr_sb = big_p.tile([128, SPC], f32)
            nc.vector.reciprocal_approx_accurate(
                out=recip_sb[:], in_=recip_sb[:], scratch=rscr_sb[:]
            )

            a_sb = big_p.tile([128, SPC], f32)

            # ---- main loop over windows ----
            def emit_body():
                _emit_main_loop(
                    nc, p, epool, ohpool, outp, psw_p, pso_p,
                    edges_t, out_t, iota_sb, lidx_sb, xt0_sb, xt1_sb,
                    recip_sb, a_sb, wp_sb, ws0_sb, ws1_sb, bias_sb, ones_sb,
                )

            if repeat == 1:
                emit_body()
            else:
                with tc.For_i(
                    0,
                    repeat,
                    1,
                    hint_engines=(
                        mybir.EngineType.PE,
                        mybir.EngineType.DVE,
                        mybir.EngineType.Activation,
                        mybir.EngineType.SP,
                        mybir.EngineType.Pool,
                    ),
                ):
                    emit_body()

    nc.compile()
    return nc


def _emit_main_loop(
    nc, p, epool, ohpool, outp, psw_p, pso_p,
    edges_t, out_t, iota_sb, lidx_sb, xt0_sb, xt1_sb,
    recip_sb, a_sb, wp_sb, ws0_sb, ws1_sb, bias_sb, ones_sb,
):
    f32 = mybir.dt.float32
    bf16 = mybir.dt.bfloat16

    def mmdt(ap):
        return ap.bitcast(_F32R) if USE_F32R else ap

    if True:
        if True:
            c_global = 0
            etile = None
            for w in range(p.wpc):
                kw = int(p.K[w])
                psum_w = psw_p.tile([128, WIN], f32)
                for k in range(kw):
                    c = c_global + k
                    g, j = divmod(c, 8)
                    if j == 0 or etile is None:
                        etile = epool.tile([128, 8 * 2 * CE], bf16)
                        nc.sync.dma_start(etile[:], edges_t[g])
                    oh = ohpool.tile([128, WIN], bf16)
                    nc.vector.tensor_scalar(
                        oh[:],
                        iota_sb[:],
                        lidx_sb[:, c : c + 1],
                        None,
                        mybir.AluOpType.is_equal,
                    )
                    nc.tensor.matmul(
                        psum_w[:],
                        lhsT=etile[:, ts(j, 2 * CE)],
                        rhs=oh[:],
                        start=(k == 0),
                        stop=(k == kw - 1),
                    )
                c_global += kw

                # scale by 1/D into the persistent (hi|lo)-stacked aggregate;
                # hi+lo folding happens inside the final matmul (K=128, Wp
                # stacked twice).
                wsl = ts(w, WIN)
                nc.vector.tensor_mul(a_sb[:, wsl], psum_w[:], recip_sb[:, wsl])

                # final linears for this window
                psum_o = pso_p.tile([128, CO], f32)
                nc.tensor.matmul(
                    psum_o[:], lhsT=mmdt(a_sb[:, wsl]), rhs=mmdt(wp_sb[:]),
                    start=True, stop=False,
                )
                nc.tensor.matmul(
                    psum_o[:], lhsT=mmdt(xt0_sb[:, wsl]), rhs=mmdt(ws0_sb[:]),
                    start=False, stop=False,
                )
                nc.tensor.matmul(
                    psum_o[:], lhsT=mmdt(xt1_sb[:, wsl]), rhs=mmdt(ws1_sb[:]),
                    start=False, stop=False,
                )
                nc.tensor.matmul(
                    psum_o[:], lhsT=mmdt(ones_sb[:]), rhs=mmdt(bias_sb[:]),
                    start=False, stop=True,
                )
                out_sb = outp.tile([128, CO], f32)
                nc.scalar.activation(
                    out_sb[:], psum_o[:], mybir.ActivationFunctionType.Relu
                )
                nc.sync.dma_start(out_t[wsl, :], out_sb[:])


_CACHE = {}
LAST_RESULTS = None


def _get_nc(plan):
    key = (plan.n_nodes, plan.n_cores, tuple(plan.K.tolist()))
    if key not in _CACHE:
        _CACHE[key] = _build_nc(plan)
    return _CACHE[key]


def kernel(**inputs):
    row_idx = np.asarray(inputs["row_idx"])
    plan = _make_plan(row_idx, N_NODES, N_CORES)
    in_maps = _prep_inputs(inputs, plan)
    nc = _get_nc(plan)
    res = run_bass_kernel_spmd(nc, in_maps, core_ids=list(range(plan.n_cores)))
    global LAST_RESULTS
    LAST_RESULTS = res
    out_all = np.concatenate([r["out"] for r in res.results], axis=0)
    valid = plan.slot_node >= 0
    out = np.empty((N_NODES, CO), np.float32)
    out[plan.slot_node[valid]] = out_all[valid]
    return out


# revision 39
# speedup vs baseline: 1.7343x; 1.7343x over previous
# GCN-conv kernel for Trainium2 (Bass/Tile), 8-core SPMD.
#
# Reference computation:
#   A   = segment_sum(edge_attr[E,64], row_idx, N)          # scatter-add
#   out = relu((A / D[:,None]) @ W_pass + b_pass + X @ W_self + b_self)
#
# Sharding strategy (host-side prep, hardcoded for the 50000/1.6M problem):
#   * Nodes are assigned to 8*49 = 392 bins of <=128 "lanes" each via a
#     degree-balanced snake assignment, so each bin owns ~4080 edges.
#     Bin b -> (core b//49, window b%49).  Each core owns 49 windows
#     (6272 node slots) and ONLY the edges targeting its own nodes, so no
#     cross-core reduction is needed.
#   * Edges are bucketed by bin.  Each 128-edge chunk targets a single
#     128-lane window.  Per chunk the device builds a one-hot matrix
#     onehot[e, lane] = (lidx[e] == lane) on the vector engine and the
#     tensor engine computes  edge_chunk.T @ onehot  accumulated in PSUM
#     over the window's chunks -> per-window aggregate [64, 128].
#   * fp32 edge values are split on the host into bf16 hi + bf16 lo parts
#     (exact to ~2^-17), laid out as 128 stationary columns (64 hi | 64 lo)
#     so ONE bf16 matmul per chunk computes both partial sums at full PE
#     rate; the two halves are folded (and scaled by 1/D) when copying
#     PSUM -> SBUF.
#   * X is pre-transposed on the host; final linears run per window:
#     psum[128n, 256] = A_w.T@W_pass + XT_w.T@W_self + 1.T@bias, then ReLU
#     on the scalar engine and a direct DMA store.
#   * Output rows come back in slot order and are un-permuted on the host.

import math
import os
import tempfile
from types import SimpleNamespace

import numpy as np
import ml_dtypes

import concourse.bass as bass
import concourse.mybir as mybir
import concourse.tile as tile
from concourse import bacc
from concourse.bass import ts
from concourse.bass_utils import run_bass_kernel_spmd

BF16 = ml_dtypes.bfloat16

# problem dims (hardcoded per contract)
N_NODES = 50000
N_EDGES = 1600000
CE = 64
CN = 256
CO = 256
N_CORES = 8
WIN = 128  # nodes per window (PSUM free dim of the segment matmul)

# final-linear matmul dtype: fp32 is exact but 4 cyc/row on PE; float32r is
# full rate at N>=256 but with hardware-specific numerics (validated on HW
# before enabling).
USE_F32R = False
LAG_FINALS = True
# 1-in-N one-hot builds routed to the scalar engine (0 = all on vector)
ACT_OH_EVERY = 7

_F32R = mybir.dt.float32r


def _make_plan(row_idx, n_nodes, n_cores):
    """Host-side layout plan. Pure numpy, no device work."""
    row_idx = np.asarray(row_idx).astype(np.int64)
    E = row_idx.shape[0]
    wpc = math.ceil(n_nodes / (WIN * n_cores))  # windows per core
    nbins = wpc * n_cores
    slots_per_core = wpc * WIN

    deg = np.bincount(row_idx, minlength=n_nodes)
    # snake assignment of nodes (sorted by degree desc) into bins ->
    # per-bin edge counts are near-equal.
    order_nodes = np.argsort(-deg, kind="stable")
    ranks = np.arange(n_nodes)
    r, j = ranks // nbins, ranks % nbins
    binid_for_rank = np.where(r % 2 == 0, j, nbins - 1 - j)
    node_bin = np.empty(n_nodes, np.int64)
    node_bin[order_nodes] = binid_for_rank

    # lane within bin
    nodes_sorted = np.argsort(node_bin, kind="stable")
    bin_ncnt = np.bincount(node_bin, minlength=nbins)
    assert bin_ncnt.max() <= WIN, f"bin overflow: {bin_ncnt.max()} nodes"
    nstarts = np.concatenate([[0], np.cumsum(bin_ncnt)[:-1]])
    lane_sorted = np.arange(n_nodes) - np.repeat(nstarts, bin_ncnt)
    node_lane = np.empty(n_nodes, np.int64)
    node_lane[nodes_sorted] = lane_sorted

    node_slot = node_bin * WIN + node_lane  # node -> global slot
    slot_node = np.full(nbins * WIN, -1, np.int64)
    slot_node[node_slot] = np.arange(n_nodes)

    # edges -> (bin, lane)
    ebin = node_bin[row_idx]
    elane = node_lane[row_idx]
    ecnt = np.bincount(ebin, minlength=nbins).reshape(n_cores, wpc)
    K = np.maximum(1, np.ceil(ecnt.max(axis=0) / 128.0)).astype(np.int64)  # [wpc]
    # pad total chunk count to a multiple of 8 (edge DMA tile = 8 chunks)
    K[-1] += (-K.sum()) % 8
    C_total = int(K.sum())
    G = C_total // 8
    caps = K * 128
    woff = np.concatenate([[0], np.cumsum(caps)[:-1]])  # chunk-slot offset per window

    # position of each edge inside its core's padded edge buffer
    eorder = np.argsort(ebin, kind="stable")
    ebin_s = ebin[eorder]
    ecnt_flat = np.bincount(ebin, minlength=nbins)
    estarts = np.concatenate([[0], np.cumsum(ecnt_flat)[:-1]])
    rank_in_bin = np.arange(E) - np.repeat(estarts, ecnt_flat)
    w_of = ebin_s % wpc
    core_of = ebin_s // wpc
    pos_in_core = woff[w_of] + rank_in_bin

    return SimpleNamespace(
        n_nodes=n_nodes,
        n_cores=n_cores,
        wpc=wpc,
        nbins=nbins,
        slots_per_core=slots_per_core,
        K=K,
        C_total=C_total,
        G=G,
        slot_node=slot_node,
        eorder=eorder,
        core_of=core_of,
        pos_in_core=pos_in_core,
        elane=elane,
    )


def _prep_inputs(inputs, plan):
    """Build the 8 per-core input maps (and shared constants)."""
    p = plan
    edge_attr = np.ascontiguousarray(np.asarray(inputs["edge_attr"], np.float32))
    X = np.asarray(inputs["X"], np.float32)
    D = np.asarray(inputs["D"], np.float32)
    Wp = np.ascontiguousarray(np.asarray(inputs["W_pass"], np.float32))
    Ws = np.ascontiguousarray(np.asarray(inputs["W_self"], np.float32))
    bp = np.asarray(inputs["b_pass"], np.float32).reshape(1, CO)
    bs = np.asarray(inputs["b_self"], np.float32).reshape(1, CO)

    EB = p.C_total * 128  # edge slots per core

    ehi = edge_attr.astype(BF16)
    elo = (edge_attr - ehi.astype(np.float32)).astype(BF16)

    src = p.eorder
    ebuf = np.zeros((p.n_cores, EB, 2 * CE), BF16)
    ebuf[p.core_of, p.pos_in_core, 0:CE] = ehi[src]
    ebuf[p.core_of, p.pos_in_core, CE : 2 * CE] = elo[src]
    # [core, chunk, lane, 128] -> [core, G, 128(lane), 8(sub), 128]
    ebuf = ebuf.reshape(p.n_cores, p.G, 8, 128, 2 * CE).transpose(0, 1, 3, 2, 4)
    ebuf = np.ascontiguousarray(ebuf).reshape(p.n_cores, p.G, 128, 8 * 2 * CE)

    lidx = np.zeros((p.n_cores, EB), np.float32)
    lidx[p.core_of, p.pos_in_core] = p.elane[src].astype(np.float32)
    lidxT = np.ascontiguousarray(
        lidx.reshape(p.n_cores, p.C_total, 128).transpose(0, 2, 1)
    )

    nslots = p.nbins * WIN
    valid = p.slot_node >= 0
    XT_all = np.zeros((CN, nslots), np.float32)
    XT_all[:, valid] = X.T[:, p.slot_node[valid]]
    XT_hi = XT_all.astype(BF16)
    XT_lo = (XT_all - XT_hi.astype(np.float32)).astype(BF16)
    Ws_hi = Ws.astype(BF16)
    Ws_lo = (Ws - Ws_hi.astype(np.float32)).astype(BF16)
    ws_pack = np.stack([Ws_hi, Ws_lo])  # [2, CN, CO]
    D_all = np.ones(nslots, np.float32)
    D_all[valid] = D[p.slot_node[valid]]

    iota_t = np.tile(np.arange(128, dtype=np.float32), (128, 1)).astype(BF16)
    # W_pass stacked twice: the final matmul contracts the (hi|lo)-stacked
    # aggregate over K=128, folding hi+lo for free.
    Wp2 = np.ascontiguousarray(np.vstack([Wp, Wp]))
    # combined bias as bf16 hi/lo rows (consumed by a K=2 ones matmul)
    bias = (bp + bs).astype(np.float32)
    bias_hi = bias.astype(BF16)
    bias_lo = (bias - bias_hi.astype(np.float32)).astype(BF16)
    bias_hilo = np.ascontiguousarray(np.vstack([bias_hi, bias_lo]))

    in_maps = []
    for c in range(p.n_cores):
        s0, s1 = c * p.slots_per_core, (c + 1) * p.slots_per_core
        db = D_all[s0:s1].reshape(1, p.slots_per_core)
        # 4 planes of [128, SPC] bf16: hi rows 0:128, hi rows 128:256,
        # lo rows 0:128, lo rows 128:256
        xt = np.stack(
            [XT_hi[0:128, s0:s1], XT_hi[128:256, s0:s1],
             XT_lo[0:128, s0:s1], XT_lo[128:256, s0:s1]]
        )
        in_maps.append(
            {
                "edges": ebuf[c],
                "lidx": lidxT[c],
                "iota": iota_t,
                "xt": np.ascontiguousarray(xt),
                "db": np.ascontiguousarray(db),
                "wp": Wp2,
                "ws": ws_pack,
                "bias": bias_hilo,
            }
        )
    return in_maps


def _build_nc(plan, repeat=1):
    """Emit the Bass/Tile program (same structure for all cores).

    repeat > 1 wraps the main loop in a hardware For loop that recomputes the
    same result `repeat` times — used only for benchmarking (amortizes the
    host->device dispatch overhead of the measurement harness).
    """
    p = plan
    SPC = p.slots_per_core
    f32 = mybir.dt.float32
    bf16 = mybir.dt.bfloat16

    nc = bacc.Bacc(
        "TRN2",
        target_bir_lowering=False,
        debug=False,
        num_devices=p.n_cores,
    )

    edges_t = nc.dram_tensor("edges", [p.G, 128, 8 * 2 * CE], bf16, kind="ExternalInput")
    lidx_t = nc.dram_tensor("lidx", [128, p.C_total], f32, kind="ExternalInput")
    iota_t = nc.dram_tensor("iota", [128, 128], bf16, kind="ExternalInput")
    xt_t = nc.dram_tensor("xt", [4, 128, SPC], bf16, kind="ExternalInput")
    db_t = nc.dram_tensor("db", [1, SPC], f32, kind="ExternalInput")
    wp_t = nc.dram_tensor("wp", [2 * CE, CO], f32, kind="ExternalInput")
    ws_t = nc.dram_tensor("ws", [2, CN, CO], bf16, kind="ExternalInput")
    bias_t = nc.dram_tensor("bias", [2, CO], bf16, kind="ExternalInput")
    out_t = nc.dram_tensor("out", [SPC, CO], f32, kind="ExternalOutput")

    with tile.TileContext(nc, num_cores=p.n_cores) as tc:
        with (
            tc.tile_pool(name="const", bufs=1) as const_p,
            tc.tile_pool(name="big", bufs=1) as big_p,
            tc.tile_pool(name="epool", bufs=4) as epool,
            tc.tile_pool(name="ohpool", bufs=12) as ohpool,
            tc.tile_pool(name="outp", bufs=4) as outp,
            tc.tile_pool(name="psw", bufs=4, space="PSUM") as psw_p,
            tc.tile_pool(name="pso", bufs=3, space="PSUM") as pso_p,
        ):
            # ---- constants / persistent tensors ----
            # ordered so the tensors gating the first windows (iota, lidx,
            # 1/D) load before the bulky X planes that only the finals need
            iota_sb = const_p.tile([128, 128], bf16)
            nc.scalar.dma_start(iota_sb[:], iota_t[:])
            lidx_sb = big_p.tile([128, p.C_total], f32)
            nc.scalar.dma_start(lidx_sb[:], lidx_t[:])
            # negated lidx: per-partition bias for the ACT-engine one-hot path
            nlidx_sb = big_p.tile([128, p.C_total], f32)
            nc.vector.tensor_scalar(
                nlidx_sb[:], lidx_sb[:], -1.0, None, mybir.AluOpType.mult
            )

            recip_sb = big_p.tile([128, SPC], f32)
            nc.scalar.dma_start(recip_sb[:], db_t[:].partition_broadcast(128))
            rscr_sb = big_p.tile([128, SPC], f32)
            nc.vector.reciprocal_approx_accurate(
                out=recip_sb[:], in_=recip_sb[:], scratch=rscr_sb[:]
            )

            wp_sb = const_p.tile([2 * CE, CO], f32)
            nc.scalar.dma_start(wp_sb[:], wp_t[:])
            # W_self hi/lo halves: [which, rows, CO]
            ws_sb = {}
            for hl in (0, 1):
                for half in (0, 1):
                    t = const_p.tile([128, CO], bf16, tag=f"ws{hl}{half}")
                    nc.scalar.dma_start(t[:], ws_t[hl, ts(half, 128), :])
                    ws_sb[(hl, half)] = t
            bias_sb = const_p.tile([2, CO], bf16)
            nc.scalar.dma_start(bias_sb[:], bias_t[:])
            ones_sb = const_p.tile([2, 128], bf16)
            nc.vector.memset(ones_sb[:], 1.0)

            xt_sb = {}
            for i in range(4):
                t = big_p.tile([128, SPC], bf16, tag=f"xt{i}")
                nc.scalar.dma_start(t[:], xt_t[i])
                xt_sb[i] = t

            a_sb = big_p.tile([128, SPC], f32)

            # ---- main loop over windows ----
            def emit_body():
                _emit_main_loop(
                    nc, p, epool, ohpool, outp, psw_p, pso_p,
                    edges_t, out_t, iota_sb, lidx_sb, nlidx_sb, xt_sb,
                    recip_sb, a_sb, wp_sb, ws_sb, bias_sb, ones_sb,
                )

            if repeat == 1:
                emit_body()
            else:
                with tc.For_i(
                    0,
                    repeat,
                    1,
                    hint_engines=(
                        mybir.EngineType.PE,
                        mybir.EngineType.DVE,
                        mybir.EngineType.Activation,
                        mybir.EngineType.SP,
                        mybir.EngineType.Pool,
                    ),
                ):
                    emit_body()

    nc.compile()
    return nc


def _emit_main_loop(
    nc, p, epool, ohpool, outp, psw_p, pso_p,
    edges_t, out_t, iota_sb, lidx_sb, nlidx_sb, xt_sb,
    recip_sb, a_sb, wp_sb, ws_sb, bias_sb, ones_sb,
):
    f32 = mybir.dt.float32
    bf16 = mybir.dt.bfloat16

    def emit_finals(w):
        # final linears for window w: out = relu(A_w/D @ Wp + X_w @ Ws + b).
        # X@Ws runs in bf16 hi/lo (hi*hi + hi*lo + lo*hi), exact to ~2^-16.
        wsl = ts(w, WIN)
        psum_o = pso_p.tile([128, CO], f32)
        nc.tensor.matmul(
            psum_o[:], lhsT=a_sb[:, wsl], rhs=wp_sb[:], start=True, stop=False,
        )
        for hl_x, hl_w in ((0, 0), (0, 1), (1, 0)):
            for half in (0, 1):
                nc.tensor.matmul(
                    psum_o[:],
                    lhsT=xt_sb[2 * hl_x + half][:, wsl],
                    rhs=ws_sb[(hl_w, half)][:],
                    start=False,
                    stop=False,
                )
        nc.tensor.matmul(
            psum_o[:], lhsT=ones_sb[:], rhs=bias_sb[:], start=False, stop=True,
        )
        out_sb = outp.tile([128, CO], f32)
        nc.scalar.activation(
            out_sb[:], psum_o[:], mybir.ActivationFunctionType.Relu
        )
        nc.sync.dma_start(out_t[wsl, :], out_sb[:])

    c_global = 0
    etile = None
    for w in range(p.wpc):
        kw = int(p.K[w])
        psum_w = psw_p.tile([128, WIN], f32)
        for k in range(kw):
            c = c_global + k
            g, j = divmod(c, 8)
            if j == 0 or etile is None:
                etile = epool.tile([128, 8 * 2 * CE], bf16)
                nc.sync.dma_start(etile[:], edges_t[g])
            oh = ohpool.tile([128, WIN], bf16)
            if ACT_OH_EVERY and (c % ACT_OH_EVERY) == ACT_OH_EVERY - 1:
                # offload to the scalar engine: relu(1 - |iota - lidx|)
                # is an exact 0/1 one-hot for integer-valued inputs
                oht = ohpool.tile([128, WIN], bf16, tag="oht")
                nc.scalar.activation(
                    oht[:], iota_sb[:], mybir.ActivationFunctionType.Abs,
                    bias=nlidx_sb[:, c : c + 1],
                )
                nc.scalar.activation(
                    oh[:], oht[:], mybir.ActivationFunctionType.Relu,
                    bias=1.0, scale=-1.0,
                )
            else:
                nc.vector.tensor_scalar(
                    oh[:],
                    iota_sb[:],
                    lidx_sb[:, c : c + 1],
                    None,
                    mybir.AluOpType.is_equal,
                )
            nc.tensor.matmul(
                psum_w[:],
                lhsT=etile[:, ts(j, 2 * CE)],
                rhs=oh[:],
                start=(k == 0),
                stop=(k == kw - 1),
            )
            # previous window's finals, emitted mid-stream so the PE never
            # stalls waiting for that window's 1/D scale on the vector engine
            if LAG_FINALS and k == min(4, kw - 1) and w > 0:
                emit_finals(w - 1)
        c_global += kw
        if not LAG_FINALS and w < p.wpc - 1:
            emit_finals(w)

        # scale by 1/D into the persistent (hi|lo)-stacked aggregate; hi+lo
        # folding happens inside the final matmul (K=128, Wp stacked twice).
        nc.vector.tensor_mul(a_sb[:, ts(w, WIN)], psum_w[:], recip_sb[:, ts(w, WIN)])

    emit_finals(p.wpc - 1)


_CACHE = {}
LAST_RESULTS = None


def _get_nc(plan):
    key = (plan.n_nodes, plan.n_cores, tuple(plan.K.tolist()))
    if key not in _CACHE:
        _CACHE[key] = _build_nc(plan)
    return _CACHE[key]


def kernel(**inputs):
    row_idx = np.asarray(inputs["row_idx"])
    plan = _make_plan(row_idx, N_NODES, N_CORES)
    in_maps = _prep_inputs(inputs, plan)
    nc = _get_nc(plan)
    res = run_bass_kernel_spmd(nc, in_maps, core_ids=list(range(plan.n_cores)))
    global LAST_RESULTS
    LAST_RESULTS = res
    out_all = np.concatenate([r["out"] for r in res.results], axis=0)
    valid = plan.slot_node >= 0
    out = np.empty((N_NODES, CO), np.float32)
    out[plan.slot_node[valid]] = out_all[valid]
    return out


# revision 47
# speedup vs baseline: 2.4398x; 1.4068x over previous
# GCN-conv kernel for Trainium2 (Bass/Tile), 8-core SPMD.
#
# Reference computation:
#   A   = segment_sum(edge_attr[E,64], row_idx, N)          # scatter-add
#   out = relu((A / D[:,None]) @ W_pass + b_pass + X @ W_self + b_self)
#
# Sharding strategy (host-side prep, hardcoded for the 50000/1.6M problem):
#   * Nodes are assigned to 8*49 = 392 bins of <=128 "lanes" each via a
#     degree-balanced snake assignment, so each bin owns ~4080 edges.
#     Bin b -> (core b//49, window b%49).  Each core owns 49 windows
#     (6272 node slots) and ONLY the edges targeting its own nodes, so no
#     cross-core reduction is needed.
#   * Edges are bucketed by bin.  Each 128-edge chunk targets a single
#     128-lane window.  Per chunk the device builds a one-hot matrix
#     onehot[e, lane] = (lidx[e] == lane) on the vector engine and the
#     tensor engine computes  edge_chunk.T @ onehot  accumulated in PSUM
#     over the window's chunks -> per-window aggregate [64, 128].
#   * fp32 edge values are split on the host into bf16 hi + bf16 lo parts
#     (exact to ~2^-17), laid out as 128 stationary columns (64 hi | 64 lo)
#     so ONE bf16 matmul per chunk computes both partial sums at full PE
#     rate; the two halves are folded (and scaled by 1/D) when copying
#     PSUM -> SBUF.
#   * X is pre-transposed on the host; final linears run per window:
#     psum[128n, 256] = A_w.T@W_pass + XT_w.T@W_self + 1.T@bias, then ReLU
#     on the scalar engine and a direct DMA store.
#   * Output rows come back in slot order and are un-permuted on the host.

import math
import os
import tempfile
from types import SimpleNamespace

import numpy as np
import ml_dtypes

import concourse.bass as bass
import concourse.mybir as mybir
import concourse.tile as tile
from concourse import bacc
from concourse.bass import ts
from concourse.bass_utils import run_bass_kernel_spmd

BF16 = ml_dtypes.bfloat16

# problem dims (hardcoded per contract)
N_NODES = 50000
N_EDGES = 1600000
CE = 64
CN = 256
CO = 256
N_CORES = 8
WIN = 128  # nodes per window (PSUM free dim of the segment matmul)

# final-linear matmul dtype: fp32 is exact but 4 cyc/row on PE; float32r is
# full rate at N>=256 but with hardware-specific numerics (validated on HW
# before enabling).
USE_F32R = False
LAG_FINALS = True
# 1-in-N one-hot builds routed to the scalar engine (0 = all on vector)
ACT_OH_EVERY = 7

_F32R = mybir.dt.float32r


def _make_plan(row_idx, n_nodes, n_cores):
    """Host-side layout plan. Pure numpy, no device work."""
    row_idx = np.asarray(row_idx).astype(np.int64)
    E = row_idx.shape[0]
    wpc = math.ceil(n_nodes / (WIN * n_cores))  # windows per core
    nbins = wpc * n_cores
    slots_per_core = wpc * WIN

    deg = np.bincount(row_idx, minlength=n_nodes)
    # snake assignment of nodes (sorted by degree desc) into bins ->
    # per-bin edge counts are near-equal.
    order_nodes = np.argsort(-deg, kind="stable")
    ranks = np.arange(n_nodes)
    r, j = ranks // nbins, ranks % nbins
    binid_for_rank = np.where(r % 2 == 0, j, nbins - 1 - j)
    node_bin = np.empty(n_nodes, np.int64)
    node_bin[order_nodes] = binid_for_rank

    # lane within bin
    nodes_sorted = np.argsort(node_bin, kind="stable")
    bin_ncnt = np.bincount(node_bin, minlength=nbins)
    assert bin_ncnt.max() <= WIN, f"bin overflow: {bin_ncnt.max()} nodes"
    nstarts = np.concatenate([[0], np.cumsum(bin_ncnt)[:-1]])
    lane_sorted = np.arange(n_nodes) - np.repeat(nstarts, bin_ncnt)
    node_lane = np.empty(n_nodes, np.int64)
    node_lane[nodes_sorted] = lane_sorted

    node_slot = node_bin * WIN + node_lane  # node -> global slot
    slot_node = np.full(nbins * WIN, -1, np.int64)
    slot_node[node_slot] = np.arange(n_nodes)

    # edges -> (bin, lane)
    ebin = node_bin[row_idx]
    elane = node_lane[row_idx]
    ecnt = np.bincount(ebin, minlength=nbins).reshape(n_cores, wpc)
    K = np.maximum(1, np.ceil(ecnt.max(axis=0) / 128.0)).astype(np.int64)  # [wpc]
    # pad total chunk count to a multiple of 8 (edge DMA tile = 8 chunks)
    K[-1] += (-K.sum()) % 8
    C_total = int(K.sum())
    G = C_total // 8
    caps = K * 128
    woff = np.concatenate([[0], np.cumsum(caps)[:-1]])  # chunk-slot offset per window

    # position of each edge inside its core's padded edge buffer
    eorder = np.argsort(ebin, kind="stable")
    ebin_s = ebin[eorder]
    ecnt_flat = np.bincount(ebin, minlength=nbins)
    estarts = np.concatenate([[0], np.cumsum(ecnt_flat)[:-1]])
    rank_in_bin = np.arange(E) - np.repeat(estarts, ecnt_flat)
    w_of = ebin_s % wpc
    core_of = ebin_s // wpc
    pos_in_core = woff[w_of] + rank_in_bin

    return SimpleNamespace(
        n_nodes=n_nodes,
        n_cores=n_cores,
        wpc=wpc,
        nbins=nbins,
        slots_per_core=slots_per_core,
        K=K,
        C_total=C_total,
        G=G,
        slot_node=slot_node,
        eorder=eorder,
        core_of=core_of,
        pos_in_core=pos_in_core,
        elane=elane,
    )


def _prep_inputs(inputs, plan):
    """Build the 8 per-core input maps (and shared constants)."""
    p = plan
    edge_attr = np.ascontiguousarray(np.asarray(inputs["edge_attr"], np.float32))
    X = np.asarray(inputs["X"], np.float32)
    D = np.asarray(inputs["D"], np.float32)
    Wp = np.ascontiguousarray(np.asarray(inputs["W_pass"], np.float32))
    Ws = np.ascontiguousarray(np.asarray(inputs["W_self"], np.float32))
    bp = np.asarray(inputs["b_pass"], np.float32).reshape(1, CO)
    bs = np.asarray(inputs["b_self"], np.float32).reshape(1, CO)

    EB = p.C_total * 128  # edge slots per core

    ehi = edge_attr.astype(BF16)
    elo = (edge_attr - ehi.astype(np.float32)).astype(BF16)

    src = p.eorder
    ebuf = np.zeros((p.n_cores, EB, 2 * CE), BF16)
    ebuf[p.core_of, p.pos_in_core, 0:CE] = ehi[src]
    ebuf[p.core_of, p.pos_in_core, CE : 2 * CE] = elo[src]
    # [core, chunk, lane, 128] -> [core, G, 128(lane), 8(sub), 128]
    ebuf = ebuf.reshape(p.n_cores, p.G, 8, 128, 2 * CE).transpose(0, 1, 3, 2, 4)
    ebuf = np.ascontiguousarray(ebuf).reshape(p.n_cores, p.G, 128, 8 * 2 * CE)

    lidx = np.zeros((p.n_cores, EB), np.float32)
    lidx[p.core_of, p.pos_in_core] = p.elane[src].astype(np.float32)
    lidxT = np.ascontiguousarray(
        lidx.reshape(p.n_cores, p.C_total, 128).transpose(0, 2, 1)
    )

    nslots = p.nbins * WIN
    valid = p.slot_node >= 0
    XT_all = np.zeros((CN, nslots), np.float32)
    XT_all[:, valid] = X.T[:, p.slot_node[valid]]
    XT_hi = XT_all.astype(BF16)
    XT_lo = (XT_all - XT_hi.astype(np.float32)).astype(BF16)
    Ws_hi = Ws.astype(BF16)
    Ws_lo = (Ws - Ws_hi.astype(np.float32)).astype(BF16)
    ws_pack = np.stack([Ws_hi, Ws_lo])  # [2, CN, CO]
    D_all = np.ones(nslots, np.float32)
    D_all[valid] = D[p.slot_node[valid]]

    iota_t = np.tile(np.arange(128, dtype=np.float32), (128, 1)).astype(BF16)
    # W_pass stacked twice: the final matmul contracts the (hi|lo)-stacked
    # aggregate over K=128, folding hi+lo for free.
    Wp2 = np.ascontiguousarray(np.vstack([Wp, Wp]))
    # combined bias as bf16 hi/lo rows (consumed by a K=2 ones matmul)
    bias = (bp + bs).astype(np.float32)
    bias_hi = bias.astype(BF16)
    bias_lo = (bias - bias_hi.astype(np.float32)).astype(BF16)
    bias_hilo = np.ascontiguousarray(np.vstack([bias_hi, bias_lo]))

    in_maps = []
    for c in range(p.n_cores):
        s0, s1 = c * p.slots_per_core, (c + 1) * p.slots_per_core
        db = D_all[s0:s1].reshape(1, p.slots_per_core)
        # 4 planes of [128, SPC] bf16: hi rows 0:128, hi rows 128:256,
        # lo rows 0:128, lo rows 128:256
        xt = np.stack(
            [XT_hi[0:128, s0:s1], XT_hi[128:256, s0:s1],
             XT_lo[0:128, s0:s1], XT_lo[128:256, s0:s1]]
        )
        in_maps.append(
            {
                "edges": ebuf[c],
                "lidx": lidxT[c],
                "iota": iota_t,
                "xt": np.ascontiguousarray(xt),
                "db": np.ascontiguousarray(db),
                "wp": Wp2,
                "ws": ws_pack,
                "bias": bias_hilo,
            }
        )
    return in_maps


def _build_nc(plan, repeat=1):
    """Emit the Bass/Tile program (same structure for all cores).

    repeat > 1 wraps the main loop in a hardware For loop that recomputes the
    same result `repeat` times — used only for benchmarking (amortizes the
    host->device dispatch overhead of the measurement harness).
    """
    p = plan
    SPC = p.slots_per_core
    f32 = mybir.dt.float32
    bf16 = mybir.dt.bfloat16

    nc = bacc.Bacc(
        "TRN2",
        target_bir_lowering=False,
        debug=False,
        num_devices=p.n_cores,
    )

    edges_t = nc.dram_tensor("edges", [p.G, 128, 8 * 2 * CE], bf16, kind="ExternalInput")
    lidx_t = nc.dram_tensor("lidx", [128, p.C_total], f32, kind="ExternalInput")
    iota_t = nc.dram_tensor("iota", [128, 128], bf16, kind="ExternalInput")
    xt_t = nc.dram_tensor("xt", [4, 128, SPC], bf16, kind="ExternalInput")
    db_t = nc.dram_tensor("db", [1, SPC], f32, kind="ExternalInput")
    wp_t = nc.dram_tensor("wp", [2 * CE, CO], f32, kind="ExternalInput")
    ws_t = nc.dram_tensor("ws", [2, CN, CO], bf16, kind="ExternalInput")
    bias_t = nc.dram_tensor("bias", [2, CO], bf16, kind="ExternalInput")
    out_t = nc.dram_tensor("out", [SPC, CO], f32, kind="ExternalOutput")

    with tile.TileContext(nc, num_cores=p.n_cores) as tc:
        with (
            tc.tile_pool(name="const", bufs=1) as const_p,
            tc.tile_pool(name="big", bufs=1) as big_p,
            tc.tile_pool(name="epool", bufs=12) as epool,
            tc.tile_pool(name="ohpool", bufs=20) as ohpool,
            tc.tile_pool(name="outp", bufs=6) as outp,
            tc.tile_pool(name="psw", bufs=4, space="PSUM") as psw_p,
            tc.tile_pool(name="pso", bufs=4, space="PSUM") as pso_p,
        ):
            # ---- constants / persistent tensors ----
            # ordered so the tensors gating the first windows (iota, lidx,
            # 1/D) load before the bulky X planes that only the finals need
            iota_sb = const_p.tile([128, 128], bf16)
            nc.scalar.dma_start(iota_sb[:], iota_t[:])
            lidx_sb = big_p.tile([128, p.C_total], f32)
            nc.scalar.dma_start(lidx_sb[:], lidx_t[:])
            # negated lidx: per-partition bias for the ACT-engine one-hot path
            nlidx_sb = big_p.tile([128, p.C_total], f32)
            nc.vector.tensor_scalar(
                nlidx_sb[:], lidx_sb[:], -1.0, None, mybir.AluOpType.mult
            )

            recip_sb = big_p.tile([128, SPC], f32)
            nc.scalar.dma_start(recip_sb[:], db_t[:].partition_broadcast(128))

            wp_sb = const_p.tile([2 * CE, CO], f32)
            nc.scalar.dma_start(wp_sb[:], wp_t[:])
            # W_self hi/lo halves: [which, rows, CO]
            ws_sb = {}
            for hl in (0, 1):
                for half in (0, 1):
                    t = const_p.tile([128, CO], bf16, tag=f"ws{hl}{half}")
                    nc.scalar.dma_start(t[:], ws_t[hl, ts(half, 128), :])
                    ws_sb[(hl, half)] = t
            bias_sb = const_p.tile([2, CO], bf16)
            nc.scalar.dma_start(bias_sb[:], bias_t[:])
            ones_sb = const_p.tile([2, 128], bf16)
            nc.vector.memset(ones_sb[:], 1.0)

            xt_sb = {}
            for i in range(4):
                t = big_p.tile([128, SPC], bf16, tag=f"xt{i}")
                nc.scalar.dma_start(t[:], xt_t[i])
                xt_sb[i] = t

            a_sb = big_p.tile([128, SPC], f32)
            # a_sb doubles as the reciprocal's scratch before any window
            # writes it (the scheduler serializes via the WAR dependency)
            nc.vector.reciprocal_approx_accurate(
                out=recip_sb[:], in_=recip_sb[:], scratch=a_sb[:]
            )

            # ---- main loop over windows ----
            def emit_body():
                _emit_main_loop(
                    nc, p, epool, ohpool, outp, psw_p, pso_p,
                    edges_t, out_t, iota_sb, lidx_sb, nlidx_sb, xt_sb,
                    recip_sb, a_sb, wp_sb, ws_sb, bias_sb, ones_sb,
                )

            if repeat == 1:
                emit_body()
            else:
                with tc.For_i(
                    0,
                    repeat,
                    1,
                    hint_engines=(
                        mybir.EngineType.PE,
                        mybir.EngineType.DVE,
                        mybir.EngineType.Activation,
                        mybir.EngineType.SP,
                        mybir.EngineType.Pool,
                    ),
                ):
                    emit_body()

    nc.compile()
    return nc


def _emit_main_loop(
    nc, p, epool, ohpool, outp, psw_p, pso_p,
    edges_t, out_t, iota_sb, lidx_sb, nlidx_sb, xt_sb,
    recip_sb, a_sb, wp_sb, ws_sb, bias_sb, ones_sb,
):
    f32 = mybir.dt.float32
    bf16 = mybir.dt.bfloat16

    def emit_finals(w):
        # final linears for window w: out = relu(A_w/D @ Wp + X_w @ Ws + b).
        # X@Ws runs in bf16 hi/lo (hi*hi + hi*lo + lo*hi), exact to ~2^-16.
        wsl = ts(w, WIN)
        psum_o = pso_p.tile([128, CO], f32)
        nc.tensor.matmul(
            psum_o[:], lhsT=a_sb[:, wsl], rhs=wp_sb[:], start=True, stop=False,
        )
        for hl_x, hl_w in ((0, 0), (0, 1), (1, 0)):
            for half in (0, 1):
                nc.tensor.matmul(
                    psum_o[:],
                    lhsT=xt_sb[2 * hl_x + half][:, wsl],
                    rhs=ws_sb[(hl_w, half)][:],
                    start=False,
                    stop=False,
                )
        nc.tensor.matmul(
            psum_o[:], lhsT=ones_sb[:], rhs=bias_sb[:], start=False, stop=True,
        )
        out_sb = outp.tile([128, CO], f32)
        nc.scalar.activation(
            out_sb[:], psum_o[:], mybir.ActivationFunctionType.Relu
        )
        # store on the ACT queue: keeps the SP queue dedicated to edge tiles
        nc.scalar.dma_start(out_t[wsl, :], out_sb[:])

    c_global = 0
    etile = None
    for w in range(p.wpc):
        kw = int(p.K[w])
        psum_w = psw_p.tile([128, WIN], f32)
        for k in range(kw):
            c = c_global + k
            g, j = divmod(c, 8)
            if j == 0 or etile is None:
                etile = epool.tile([128, 8 * 2 * CE], bf16)
                nc.sync.dma_start(etile[:], edges_t[g])
            oh = ohpool.tile([128, WIN], bf16)
            if ACT_OH_EVERY and (c % ACT_OH_EVERY) == ACT_OH_EVERY - 1:
                # offload to the scalar engine: relu(1 - |iota - lidx|)
                # is an exact 0/1 one-hot for integer-valued inputs
                oht = ohpool.tile([128, WIN], bf16, tag="oht")
                nc.scalar.activation(
                    oht[:], iota_sb[:], mybir.ActivationFunctionType.Abs,
                    bias=nlidx_sb[:, c : c + 1],
                )
                nc.scalar.activation(
                    oh[:], oht[:], mybir.ActivationFunctionType.Relu,
                    bias=1.0, scale=-1.0,
                )
            else:
                nc.vector.tensor_scalar(
                    oh[:],
                    iota_sb[:],
                    lidx_sb[:, c : c + 1],
                    None,
                    mybir.AluOpType.is_equal,
                )
            nc.tensor.matmul(
                psum_w[:],
                lhsT=etile[:, ts(j, 2 * CE)],
                rhs=oh[:],
                start=(k == 0),
                stop=(k == kw - 1),
            )
            # previous window's finals, emitted mid-stream so the PE never
            # stalls waiting for that window's 1/D scale on the vector engine
            if LAG_FINALS and k == min(4, kw - 1) and w > 0:
                emit_finals(w - 1)
        c_global += kw
        if not LAG_FINALS and w < p.wpc - 1:
            emit_finals(w)

        # scale by 1/D into the persistent (hi|lo)-stacked aggregate; hi+lo
        # folding happens inside the final matmul (K=128, Wp stacked twice).
        nc.vector.tensor_mul(a_sb[:, ts(w, WIN)], psum_w[:], recip_sb[:, ts(w, WIN)])

    emit_finals(p.wpc - 1)


_CACHE = {}
LAST_RESULTS = None


def _get_nc(plan):
    key = (plan.n_nodes, plan.n_cores, tuple(plan.K.tolist()))
    if key not in _CACHE:
        _CACHE[key] = _build_nc(plan)
    return _CACHE[key]


def kernel(**inputs):
    row_idx = np.asarray(inputs["row_idx"])
    plan = _make_plan(row_idx, N_NODES, N_CORES)
    in_maps = _prep_inputs(inputs, plan)
    nc = _get_nc(plan)
    res = run_bass_kernel_spmd(nc, in_maps, core_ids=list(range(plan.n_cores)))
    global LAST_RESULTS
    LAST_RESULTS = res
    out_all = np.concatenate([r["out"] for r in res.results], axis=0)
    valid = plan.slot_node >= 0
    out = np.empty((N_NODES, CO), np.float32)
    out[plan.slot_node[valid]] = out_all[valid]
    return out
